# revision 4
# baseline (speedup 1.0000x reference)
"""Trainium2 Bass kernel: 3-level threshold activation (elementwise).

  x <  0.33          -> f32(0.333333333)  (= f32 1/3)
  0.33 <= x < 0.66   -> f32(0.6666666666) (= f32 2/3)
  x >= 0.66          -> 1.0

Exact computation in 3 elementwise passes, one per engine:
  POOL: g = (x is_ge t1) + 1          in {1, 2}
  ACT:  m = Copy(A * g)               in {A, 2A} (both exact f32)
  DVE:  out = max((x is_ge t2), m)    in {A, 2A, 1.0}

Sharding: 8192 rows split evenly across 8 NeuronCores (pure data parallel).
"""

import numpy as np

import concourse.bacc as bacc
import concourse.bass as bass
import concourse.tile as tile
from concourse import mybir
from concourse.bass_utils import run_bass_kernel_spmd

N_CORES = 8
ROWS, COLS = 8192, 8192
SHARD_ROWS = ROWS // N_CORES  # 1024
P = 128  # SBUF partitions

T1 = 0.33
T2 = 0.66
LEVEL_LO = float(np.float32(0.333333333))

_BUILT = {}


def build_nc(shard_rows: int = SHARD_ROWS, cols: int = COLS, free: int = 4096,
             bufs: int = 3) -> bass.Bass:
    nc = bacc.Bacc(
        "TRN2",
        target_bir_lowering=False,
        debug=False,
        num_devices=N_CORES,
    )
    x = nc.dram_tensor("inputs", [shard_rows, cols], mybir.dt.float32,
                       kind="ExternalInput").ap()
    o = nc.dram_tensor("out", [shard_rows, cols], mybir.dt.float32,
                       kind="ExternalOutput").ap()

    with tile.TileContext(nc) as tc:
        with tc.tile_pool(name="xp", bufs=bufs) as xp, \
             tc.tile_pool(name="gp", bufs=bufs) as gp, \
             tc.tile_pool(name="mp", bufs=bufs) as mp, \
             tc.tile_pool(name="op", bufs=bufs) as op:
            for r in range(shard_rows // P):
                for c in range(cols // free):
                    cs = slice(c * free, (c + 1) * free)
                    rs = slice(r * P, (r + 1) * P)
                    xt = xp.tile([P, free], mybir.dt.float32)
                    nc.sync.dma_start(out=xt[:], in_=x[rs, cs])
                    gt = gp.tile([P, free], mybir.dt.float32)
                    nc.gpsimd.tensor_scalar(
                        gt[:], xt[:], T1, 1.0,
                        mybir.AluOpType.is_ge, mybir.AluOpType.add)
                    mt = mp.tile([P, free], mybir.dt.float32)
                    nc.scalar.activation(
                        mt[:], gt[:], mybir.ActivationFunctionType.Copy,
                        bias=0.0, scale=LEVEL_LO)
                    ot = op.tile([P, free], mybir.dt.float32)
                    nc.vector.scalar_tensor_tensor(
                        ot[:], xt[:], T2, mt[:],
                        mybir.AluOpType.is_ge, mybir.AluOpType.max)
                    nc.sync.dma_start(out=o[rs, cs], in_=ot[:])
    nc.compile()
    return nc


def _get_nc():
    if "nc" not in _BUILT:
        _BUILT["nc"] = build_nc()
    return _BUILT["nc"]


def kernel(inputs: np.ndarray, _trace: bool = False):
    assert inputs.shape == (ROWS, COLS) and inputs.dtype == np.float32
    nc = _get_nc()
    in_maps = [
        {"inputs": np.ascontiguousarray(
            inputs[i * SHARD_ROWS:(i + 1) * SHARD_ROWS])}
        for i in range(N_CORES)
    ]
    res = run_bass_kernel_spmd(nc, in_maps, list(range(N_CORES)), trace=_trace)
    out = np.concatenate([res.results[i]["out"] for i in range(N_CORES)], axis=0)
    if _trace:
        return out, res
    return out


# revision 5
# speedup vs baseline: 5.0810x; 5.0810x over previous
"""Trainium2 Bass kernel: 3-level threshold activation (elementwise).

  x <  0.33          -> f32(0.333333333)  (= f32 1/3)
  0.33 <= x < 0.66   -> f32(0.6666666666) (= f32 2/3)
  x >= 0.66          -> 1.0

Exact computation in 3 elementwise passes, one per engine:
  POOL: g = (x is_ge t1) + 1          in {1, 2}
  ACT:  m = Copy(A * g)               in {A, 2A} (both exact f32)
  DVE:  out = max((x is_ge t2), m)    in {A, 2A, 1.0}

Sharding: 8192 rows split evenly across 8 NeuronCores (pure data parallel).
"""

import numpy as np

import concourse.bacc as bacc
import concourse.bass as bass
import concourse.tile as tile
from concourse import mybir
from concourse.bass_utils import run_bass_kernel_spmd

N_CORES = 8
ROWS, COLS = 8192, 8192
SHARD_ROWS = ROWS // N_CORES  # 1024
P = 128  # SBUF partitions

T1 = 0.33
T2 = 0.66
LEVEL_LO = float(np.float32(0.333333333))

_BUILT = {}


def build_nc(shard_rows: int = SHARD_ROWS, cols: int = COLS, free: int = 4096,
             bufs: int = 3) -> bass.Bass:
    nc = bacc.Bacc(
        "TRN2",
        target_bir_lowering=False,
        debug=False,
        num_devices=N_CORES,
    )
    x = nc.dram_tensor("inputs", [shard_rows, cols], mybir.dt.float32,
                       kind="ExternalInput").ap()
    o = nc.dram_tensor("out", [shard_rows, cols], mybir.dt.float32,
                       kind="ExternalOutput").ap()

    with tile.TileContext(nc) as tc:
        with tc.tile_pool(name="xp", bufs=bufs) as xp, \
             tc.tile_pool(name="gp", bufs=bufs) as gp, \
             tc.tile_pool(name="mp", bufs=bufs) as mp, \
             tc.tile_pool(name="op", bufs=bufs) as op:
            for r in range(shard_rows // P):
                for c in range(cols // free):
                    cs = slice(c * free, (c + 1) * free)
                    rs = slice(r * P, (r + 1) * P)
                    xt = xp.tile([P, free], mybir.dt.float32)
                    nc.sync.dma_start(out=xt[:], in_=x[rs, cs])
                    gt = gp.tile([P, free], mybir.dt.float32)
                    nc.vector.tensor_scalar(
                        gt[:], xt[:], T1, 1.0,
                        mybir.AluOpType.is_ge, mybir.AluOpType.add)
                    mt = mp.tile([P, free], mybir.dt.float32)
                    nc.scalar.activation(
                        mt[:], gt[:], mybir.ActivationFunctionType.Copy,
                        bias=0.0, scale=LEVEL_LO)
                    ot = op.tile([P, free], mybir.dt.float32)
                    nc.vector.scalar_tensor_tensor(
                        ot[:], xt[:], T2, mt[:],
                        mybir.AluOpType.is_ge, mybir.AluOpType.max)
                    nc.sync.dma_start(out=o[rs, cs], in_=ot[:])
    nc.compile()
    return nc


def _get_nc():
    if "nc" not in _BUILT:
        _BUILT["nc"] = build_nc()
    return _BUILT["nc"]


def kernel(inputs: np.ndarray, _trace: bool = False):
    assert inputs.shape == (ROWS, COLS) and inputs.dtype == np.float32
    nc = _get_nc()
    in_maps = [
        {"inputs": np.ascontiguousarray(
            inputs[i * SHARD_ROWS:(i + 1) * SHARD_ROWS])}
        for i in range(N_CORES)
    ]
    res = run_bass_kernel_spmd(nc, in_maps, list(range(N_CORES)), trace=_trace)
    out = np.concatenate([res.results[i]["out"] for i in range(N_CORES)], axis=0)
    if _trace:
        return out, res
    return out


# revision 6
# speedup vs baseline: 5.3122x; 1.0455x over previous
"""Trainium2 Bass kernel: 3-level threshold activation (elementwise).

  x <  0.33          -> f32(0.333333333)  (= f32 1/3)
  0.33 <= x < 0.66   -> f32(0.6666666666) (= f32 2/3)
  x >= 0.66          -> 1.0

Exact computation in 3 elementwise passes (all output levels land exactly
after f32 rounding, so the result is bit-identical to the jnp reference):
  DVE: g = (x is_ge t1) + 1           in {1, 2}
  ACT: m = Copy(A * g)                in {A, 2A}
  DVE: out = max((x is_ge t2), m)     in {A, 2A, 1.0}

Sharding: 8192 rows split evenly across 8 NeuronCores (pure data parallel).
Memory-bound: 67.1 MB HBM traffic per core at ~358 GB/s/core.
"""

import numpy as np

import concourse.bacc as bacc
import concourse.tile as tile
from concourse import mybir
from concourse.bass_utils import run_bass_kernel_spmd

N_CORES = 8
ROWS, COLS = 8192, 8192
SHARD_ROWS = ROWS // N_CORES  # 1024
P = 128  # SBUF partitions

T1 = 0.33
T2 = 0.66
LEVEL_LO = float(np.float32(0.333333333))

_BUILT = {}


def build_nc(shard_rows: int = SHARD_ROWS, cols: int = COLS, free: int = 4096,
             bufs: int = 3, store_engine: str = "scalar"):
    nc = bacc.Bacc(
        "TRN2",
        target_bir_lowering=False,
        debug=False,
        num_devices=N_CORES,
    )
    x = nc.dram_tensor("inputs", [shard_rows, cols], mybir.dt.float32,
                       kind="ExternalInput").ap()
    o = nc.dram_tensor("out", [shard_rows, cols], mybir.dt.float32,
                       kind="ExternalOutput").ap()
    store_eng = {"scalar": nc.scalar, "sync": nc.sync}[store_engine]

    with tile.TileContext(nc) as tc:
        with tc.tile_pool(name="xp", bufs=bufs) as xp, \
             tc.tile_pool(name="gp", bufs=bufs) as gp, \
             tc.tile_pool(name="mp", bufs=bufs) as mp, \
             tc.tile_pool(name="op", bufs=bufs) as op:
            for r in range(shard_rows // P):
                for c in range(cols // free):
                    cs = slice(c * free, (c + 1) * free)
                    rs = slice(r * P, (r + 1) * P)
                    xt = xp.tile([P, free], mybir.dt.float32)
                    nc.sync.dma_start(out=xt[:], in_=x[rs, cs])
                    gt = gp.tile([P, free], mybir.dt.float32)
                    nc.vector.tensor_scalar(
                        gt[:], xt[:], T1, 1.0,
                        mybir.AluOpType.is_ge, mybir.AluOpType.add)
                    mt = mp.tile([P, free], mybir.dt.float32)
                    nc.scalar.activation(
                        mt[:], gt[:], mybir.ActivationFunctionType.Copy,
                        bias=0.0, scale=LEVEL_LO)
                    ot = op.tile([P, free], mybir.dt.float32)
                    nc.vector.scalar_tensor_tensor(
                        ot[:], xt[:], T2, mt[:],
                        mybir.AluOpType.is_ge, mybir.AluOpType.max)
                    store_eng.dma_start(out=o[rs, cs], in_=ot[:])
    nc.compile()
    return nc


def _get_nc():
    if "nc" not in _BUILT:
        _BUILT["nc"] = build_nc()
    return _BUILT["nc"]


def kernel(inputs: np.ndarray, _trace: bool = False, _nc=None):
    assert inputs.shape == (ROWS, COLS) and inputs.dtype == np.float32
    nc = _nc if _nc is not None else _get_nc()
    in_maps = [
        {"inputs": np.ascontiguousarray(
            inputs[i * SHARD_ROWS:(i + 1) * SHARD_ROWS])}
        for i in range(N_CORES)
    ]
    res = run_bass_kernel_spmd(nc, in_maps, list(range(N_CORES)), trace=_trace)
    out = np.concatenate([res.results[i]["out"] for i in range(N_CORES)], axis=0)
    if _trace:
        return out, res
    return out
